# revision 23
# baseline (speedup 1.0000x reference)
"""CrossAttention Trainium2 kernel, SPMD over 8 NeuronCores.

Problem: x[4,2048,1024], context[4,1024,768], Wq[1024,512], Wk/Wv[768,512],
Wout[512,1024], bout[1024] -> out[4,2048,1024] (f32).

Sharding: 8 cores = 4 batches x 2 halves of the query dim n (2048 -> 2x1024).
Each core computes full attention for its (batch, n-half) with no collectives.

v2 layout/schedule (vs v1):
  - sim: both heads of a pair write disjoint 512-col halves of ONE
    [128,1024] psum tile, j0 on PE rows 0-63 / j1 on rows 64-127 (row
    tiling). Co-ready -> concurrent, halving sim PE time. One big exp
    (ScalarE) per tile.
  - attn@v: no ones column (M=64 per head); the two heads of a pair are
    COLUMN-tiled at tile_position (0,0)/(0,64) into one [128,512] psum ->
    ~2x. Softmax denominators via M=1 matmuls (lhsT=ones[128,1]) 4-way
    column-packed (4 heads/round) accumulating at psum rows {0,32,64,96}.
  - normalization: DVE reciprocal of the denominator psum; GpSimd
    partition_broadcast spreads recip rows over 64 partitions (replaces
    the v1 PE broadcast matmuls); DVE multiply -> oT bf16.
  - emission is a fine-grained software pipeline: per exp-slot the PE
    queue gets [sim(mc) | attnv(mc-1) | denom(mc-1) | proj filler MMs] so
    the in-order PE never stalls long while ScalarE (68.6us of exp, the
    serial floor) paces the middle of the kernel.
f32 accumulation everywhere (PSUM); bf16 operands for matmuls.
"""

import numpy as np
import ml_dtypes
from collections import deque

import concourse.bass as bass
import concourse.mybir as mybir
import concourse.tile as tile
from concourse import bacc
from concourse.bass_utils import run_bass_kernel_spmd

BF16 = mybir.dt.bfloat16
F32 = mybir.dt.float32

B, N, QD = 4, 2048, 1024
M, CD = 1024, 768
H, D = 8, 64
INNER = H * D  # 512
NSH = N // 2  # 1024 query rows per core
P = 128
FB = 512  # free-dim block (psum bank = 512 f32)

KQ = QD // P  # 8 contraction tiles for q-proj
KC = CD // P  # 6 contraction tiles for k/v-proj
MI = INNER // P  # 4 pairs (head pairs / inner p-tiles)
MC = M // P  # 8 m chunks
KO = INNER // P  # 4 contraction tiles for out-proj
QT = QD // P  # 8 out-proj row tiles


def build_nc(debug=False):
    nc = bacc.Bacc(None)

    xT_d = nc.declare_dram_parameter("xT", [QD, NSH], BF16, isOutput=False)
    ctxT_d = nc.declare_dram_parameter("ctxT", [CD, M], BF16, isOutput=False)
    Wq_d = nc.declare_dram_parameter("Wq", [QD, INNER], BF16, isOutput=False)
    Wk_d = nc.declare_dram_parameter("Wk", [CD, INNER], BF16, isOutput=False)
    Wv_d = nc.declare_dram_parameter("Wv", [CD, INNER], BF16, isOutput=False)
    Wout_d = nc.declare_dram_parameter("Wout", [INNER, QD], BF16, isOutput=False)
    bout_d = nc.declare_dram_parameter("bout", [QT, P, 1], F32, isOutput=False)
    outT_d = nc.declare_dram_parameter("outT", [QD, NSH], BF16, isOutput=True)
    if debug:
        dbg_qT = nc.declare_dram_parameter("dbg_qT", [MI, P, NSH], BF16, isOutput=True)
        dbg_kT = nc.declare_dram_parameter("dbg_kT", [MI, P, M], BF16, isOutput=True)
        dbg_exp = nc.declare_dram_parameter(
            "dbg_exp", [MC, P, 2 * FB], BF16, isOutput=True
        )
        dbg_oT = nc.declare_dram_parameter("dbg_oT", [MI, P, NSH], BF16, isOutput=True)

    from contextlib import ExitStack

    with tile.TileContext(nc) as tc, ExitStack() as ctx:
        persist = ctx.enter_context(tc.tile_pool(name="persist", bufs=1))
        # PSUM (8 banks): sim 2x[128,1024]=4, av/dn 2x[128,512]=2, mm 2x[128,512]=2
        pp_sim = ctx.enter_context(tc.tile_pool(name="pp_sim", bufs=2, space="PSUM"))
        pp_av = ctx.enter_context(tc.tile_pool(name="pp_av", bufs=2, space="PSUM"))
        pp_mm = ctx.enter_context(tc.tile_pool(name="pp_mm", bufs=2, space="PSUM"))
        sb_misc = ctx.enter_context(tc.tile_pool(name="sb_misc", bufs=3))
        sb_rec = ctx.enter_context(tc.tile_pool(name="sb_rec", bufs=4))
        exp_pool = ctx.enter_context(tc.tile_pool(name="expp", bufs=36))

        # ---- input DMAs (priority order: xT nb0 halves + Wq + ctxT + Wk
        # first so q/k-proj of pair 0 starts earliest) ----
        xT_sb = []
        for k in range(KQ):
            t = persist.tile([P, NSH], BF16, tag=f"xT{k}", name=f"xT{k}")
            nc.sync.dma_start(out=t[:, 0:FB], in_=xT_d[k * P : (k + 1) * P, 0:FB])
            xT_sb.append(t)
        Wq_sb = []
        for k in range(KQ):
            t2 = persist.tile([P, INNER], BF16, tag=f"Wq{k}", name=f"Wq{k}")
            nc.sync.dma_start(out=t2[:], in_=Wq_d[k * P : (k + 1) * P, :])
            Wq_sb.append(t2)
        ctxT_sb = []
        for k in range(KC):
            t = persist.tile([P, M], BF16, tag=f"ctxT{k}", name=f"ctxT{k}")
            nc.sync.dma_start(out=t[:], in_=ctxT_d[k * P : (k + 1) * P, :])
            ctxT_sb.append(t)
        Wk_sb = []
        for k in range(KC):
            t = persist.tile([P, INNER], BF16, tag=f"Wk{k}", name=f"Wk{k}")
            nc.sync.dma_start(out=t[:], in_=Wk_d[k * P : (k + 1) * P, :])
            Wk_sb.append(t)
        for k in range(KQ):
            nc.sync.dma_start(
                out=xT_sb[k][:, FB:NSH], in_=xT_d[k * P : (k + 1) * P, FB:NSH]
            )
        Wv_sb = []
        for k in range(KC):
            t2 = persist.tile([P, INNER], BF16, tag=f"Wv{k}", name=f"Wv{k}")
            nc.sync.dma_start(out=t2[:], in_=Wv_d[k * P : (k + 1) * P, :])
            Wv_sb.append(t2)
        Wout_sb = []
        for k in range(KO):
            t = persist.tile([P, QD], BF16, tag=f"Wout{k}", name=f"Wout{k}")
            nc.sync.dma_start(out=t[:], in_=Wout_d[k * P : (k + 1) * P, :])
            Wout_sb.append(t)
        bout_sb = []
        for k in range(QT):
            t = persist.tile([P, 1], F32, tag=f"bout{k}", name=f"bout{k}")
            nc.sync.dma_start(out=t[:], in_=bout_d[k])
            bout_sb.append(t)

        ones_sb = persist.tile([P, 1], BF16, tag="ones", name="ones_sb")
        nc.vector.memset(ones_sb[:], 1.0)
        ones64 = persist.tile([P, D], BF16, tag="ones64", name="ones64")
        nc.vector.memset(ones64[:], 1.0)

        # PE warm-up: p-state ramp needs sustained activity; run dummy
        # matmuls on memset data while the input DMAs stream.
        warm_w = persist.tile([P, FB], BF16, tag="warm", name="warm_w")
        nc.vector.memset(warm_w[:], 0.0)
        ps_w = pp_mm.tile([P, FB], F32, tag="mm", name="ps_w")
        for _ in range(22):
            nc.tensor.matmul(ps_w[:], warm_w[:, 0:P], warm_w[:], start=True, stop=True)
        warm_anchor = persist.tile([1, 1], F32, tag="warm_a", name="warm_anchor")
        nc.vector.tensor_copy(warm_anchor[:], ps_w[0:1, 0:1])

        vext_sb = []
        for i in range(MC):
            t = persist.tile([P, H, D + 1], BF16, tag=f"vext{i}", name=f"vext{i}")
            nc.vector.memset(t[:, :, D : D + 1], 1.0)
            vext_sb.append(t)
        qT_sb = [
            persist.tile([P, NSH], BF16, tag=f"qT{i}", name=f"qT{i}")
            for i in range(MI)
        ]
        kT_sb = [
            persist.tile([P, M], BF16, tag=f"kT{i}", name=f"kT{i}") for i in range(MI)
        ]
        oT_sb = [
            persist.tile([P, NSH], BF16, tag=f"oT{i}", name=f"oT{i}")
            for i in range(MI)
        ]

        def q_group(mi, nb):
            ps_q = pp_mm.tile([P, FB], F32, tag="mm", name="ps_q")
            for k in range(KQ):
                nc.tensor.matmul(
                    ps_q[:],
                    Wq_sb[k][:, mi * P : (mi + 1) * P],
                    xT_sb[k][:, nb * FB : (nb + 1) * FB],
                    start=(k == 0),
                    stop=(k == KQ - 1),
                )
                if k < KQ - 1:
                    yield
            nc.vector.tensor_copy(qT_sb[mi][:, nb * FB : (nb + 1) * FB], ps_q[:])
            yield

        def k_group(mi, half):
            ps_k = pp_mm.tile([P, FB], F32, tag="mm", name="ps_k")
            for k in range(KC):
                nc.tensor.matmul(
                    ps_k[:],
                    Wk_sb[k][:, mi * P : (mi + 1) * P],
                    ctxT_sb[k][:, half * FB : (half + 1) * FB],
                    start=(k == 0),
                    stop=(k == KC - 1),
                )
                if k < KC - 1:
                    yield
            nc.vector.tensor_copy(kT_sb[mi][:, half * FB : (half + 1) * FB], ps_k[:])
            yield

        def v_group(t_i):
            ps_v = pp_mm.tile([P, FB], F32, tag="mm", name="ps_v")
            for k in range(KC):
                nc.tensor.matmul(
                    ps_v[:],
                    ctxT_sb[k][:, t_i * P : (t_i + 1) * P],
                    Wv_sb[k][:],
                    start=(k == 0),
                    stop=(k == KC - 1),
                )
                if k < KC - 1:
                    yield
            nc.vector.tensor_copy(
                vext_sb[t_i][:, :, 0:D],
                ps_v[:].rearrange("p (h d) -> p h d", h=H),
            )
            yield

        def out_group(mi, nb, on_scalar):
            ps_out = pp_mm.tile([P, FB], F32, tag="mm", name="ps_out")
            for k in range(KO):
                nc.tensor.matmul(
                    ps_out[:],
                    Wout_sb[k][:, mi * P : (mi + 1) * P],
                    oT_sb[k][:, nb * FB : (nb + 1) * FB],
                    start=(k == 0),
                    stop=(k == KO - 1),
                )
                if k < KO - 1:
                    yield
            stage = sb_misc.tile([P, FB], BF16, tag="outstage", name="stage")
            if on_scalar:
                nc.scalar.add(stage[:], ps_out[:], bout_sb[mi][:])
            else:
                nc.vector.tensor_scalar_add(stage[:], ps_out[:], bout_sb[mi][:])
            nc.sync.dma_start(
                out=outT_d[mi * P : (mi + 1) * P, nb * FB : (nb + 1) * FB],
                in_=stage[:],
            )
            yield

        # ---- attention building blocks ----
        exp_t = {}

        def sim_exp(p, nb, mc):
            S = pp_sim.tile([P, 2 * FB], F32, tag="sim", name=f"S{p}_{nb}_{mc}")
            for j in range(2):
                nc.tensor.matmul(
                    S[:, j * FB : (j + 1) * FB],
                    kT_sb[p][j * D : (j + 1) * D, mc * P : (mc + 1) * P],
                    qT_sb[p][j * D : (j + 1) * D, nb * FB : (nb + 1) * FB],
                    start=True,
                    stop=True,
                    tile_position=(j * D, 0),
                )
            e = exp_pool.tile([P, 2 * FB], BF16, tag="expT", name=f"e{p}_{nb}_{mc}")
            nc.scalar.activation(
                e[:], S[:], mybir.ActivationFunctionType.Exp, scale=float(D) ** -0.5
            )
            exp_t[(p, nb, mc)] = e
            if debug and p == 0 and nb == 0:
                nc.sync.dma_start(out=dbg_exp[mc], in_=e[:])

        def attnv_round(p, nb, mc, av0, av1):
            # per-head [K=128, M=65, N=512] with the ones column producing
            # the softmax denominator in row 64 (col tiling beyond col 63
            # is broken on TRN2 — quadrant-3 XBUS bug — so no head pairing)
            for j, av in ((0, av0), (1, av1)):
                h = 2 * p + j
                nc.tensor.matmul(
                    av[:],
                    vext_sb[mc][:, h : h + 1, :],
                    exp_t[(p, nb, mc)][:, j * FB : (j + 1) * FB],
                    start=(mc == 0),
                    stop=(mc == MC - 1),
                )

        def normalize(p, nb, av0, av1):
            # sums (row 64 of each head's psum) -> bf16 sbuf rows {0, 32};
            # two K=1 broadcast matmuls ROW-packed at (0,0)/(32,0) run
            # concurrently; reciprocal after the broadcast; multiply.
            sums2 = sb_rec.tile([P, FB], BF16, tag="sums", name=f"sums{p}_{nb}")
            nc.vector.tensor_copy(sums2[0:1, :], av0[D : D + 1, :])
            nc.vector.tensor_copy(sums2[32:33, :], av1[D : D + 1, :])
            ps_rbs = []
            for j in range(2):
                ps_rb = pp_mm.tile([P, FB], F32, tag="mm", name=f"ps_rb{p}_{nb}_{j}")
                nc.tensor.matmul(
                    ps_rb[0:D, :],
                    ones64[32 * j : 32 * j + 1, :],
                    sums2[32 * j : 32 * j + 1, :],
                    start=True,
                    stop=True,
                    tile_position=(32 * j, 0),
                )
                ps_rbs.append(ps_rb)
            for j, av in ((0, av0), (1, av1)):
                rec = sb_rec.tile([P, FB], F32, tag="rec", name=f"rec{p}_{nb}_{j}")
                nc.vector.reciprocal_approx_fast(
                    out=rec[0:D, :], in_=ps_rbs[j][0:D, :]
                )
                nc.vector.tensor_mul(
                    oT_sb[p][j * D : (j + 1) * D, nb * FB : (nb + 1) * FB],
                    av[0:D, :],
                    rec[0:D, :],
                )

        def run(gen):
            for _ in gen:
                pass

        # ---- static slot schedule: whole proj groups emitted at exp-slot
        # boundaries, every group strictly BEFORE its first consumer
        # (emission order is the Tile framework's dependency horizon) ----
        slot_groups = {
            (0, 0, 0): [v_group(0), v_group(1)],
            (0, 0, 1): [v_group(2)],
            (0, 0, 2): [v_group(3)],
            (0, 0, 3): [v_group(4), k_group(0, 1)],
            (0, 0, 4): [v_group(5)],
            (0, 0, 5): [v_group(6)],
            (0, 0, 6): [v_group(7)],
            (0, 0, 7): [q_group(0, 1)],
            (0, 1, 0): [k_group(1, 0)],
            (0, 1, 2): [q_group(1, 0)],
            (0, 1, 4): [k_group(1, 1)],
            (0, 1, 6): [q_group(1, 1)],
            (1, 0, 0): [k_group(2, 0)],
            (1, 0, 3): [q_group(2, 0)],
            (1, 1, 0): [k_group(2, 1)],
            (1, 1, 3): [q_group(2, 1)],
            (2, 0, 0): [k_group(3, 0)],
            (2, 0, 3): [q_group(3, 0)],
            (2, 1, 0): [k_group(3, 1)],
            (2, 1, 3): [q_group(3, 1)],
        }
        # out-proj of the nb0 half rides in the last pair's nb1 window
        # (needs normalize(3, 0), emitted at the end of the nb0 half)
        for mi in range(QT):
            slot_groups[(3, 1, mi)] = [out_group(mi, 0, False)]

        # ---- lead-in ----
        run(k_group(0, 0))
        run(q_group(0, 0))

        # ---- attention: pair-major windows, nb-major inside ----
        for p in range(MI):
            for nb in range(2):
                av0 = pp_av.tile([D + 1, FB], F32, tag="av", name=f"av{p}_{nb}_0")
                av1 = pp_av.tile([D + 1, FB], F32, tag="av", name=f"av{p}_{nb}_1")
                for mc in range(MC):
                    sim_exp(p, nb, mc)
                    if mc >= 1:
                        attnv_round(p, nb, mc - 1, av0, av1)
                    for gen in slot_groups.get((p, nb, mc), ()):
                        run(gen)
                attnv_round(p, nb, MC - 1, av0, av1)
                normalize(p, nb, av0, av1)

        # ---- tail: out-proj nb1 ----
        if debug:
            for i in range(MI):
                nc.sync.dma_start(out=dbg_qT[i], in_=qT_sb[i][:])
                nc.sync.dma_start(out=dbg_kT[i], in_=kT_sb[i][:])
                nc.sync.dma_start(out=dbg_oT[i], in_=oT_sb[i][:])
        for mi in range(QT):
            for _ in out_group(mi, 1, True):
                pass

    nc.compile()
    return nc


_NC_CACHE = None


def _get_nc():
    global _NC_CACHE
    if _NC_CACHE is None:
        _NC_CACHE = build_nc()
    return _NC_CACHE


def make_in_maps(x, context, Wq, Wk, Wv, Wout, bout):
    bf = ml_dtypes.bfloat16
    Wq_b = np.ascontiguousarray(Wq).astype(bf)
    Wk_b = np.ascontiguousarray(Wk).astype(bf)
    Wv_b = np.ascontiguousarray(Wv).astype(bf)
    Wout_b = np.ascontiguousarray(Wout).astype(bf)
    bout_r = np.ascontiguousarray(bout, dtype=np.float32).reshape(QT, P, 1)
    in_maps = []
    for c in range(8):
        b, half = divmod(c, 2)
        xT = x[b].T[:, half * NSH : (half + 1) * NSH].astype(bf)
        ctxT = context[b].T.astype(bf)
        in_maps.append(
            {
                "xT": xT,
                "ctxT": ctxT,
                "Wq": Wq_b,
                "Wk": Wk_b,
                "Wv": Wv_b,
                "Wout": Wout_b,
                "bout": bout_r,
            }
        )
    return in_maps


def gather_out(results):
    out = np.empty((B, N, QD), dtype=np.float32)
    for c in range(8):
        b, half = divmod(c, 2)
        out[b, half * NSH : (half + 1) * NSH, :] = results[c]["outT"].astype(np.float32).T
    return out


def kernel(**inputs):
    nc = _get_nc()
    in_maps = make_in_maps(**inputs)
    res = run_bass_kernel_spmd(nc, in_maps, list(range(8)))
    return gather_out(res.results)


if __name__ == "__main__":
    rng = np.random.default_rng(0)
    ins = {
        "x": rng.standard_normal((B, N, QD), dtype=np.float32),
        "context": rng.standard_normal((B, M, CD), dtype=np.float32),
        "Wq": rng.standard_normal((QD, INNER), dtype=np.float32) / 32,
        "Wk": rng.standard_normal((CD, INNER), dtype=np.float32) / 27.7,
        "Wv": rng.standard_normal((CD, INNER), dtype=np.float32) / 27.7,
        "Wout": rng.standard_normal((INNER, QD), dtype=np.float32) / 22.6,
        "bout": rng.standard_normal((QD,), dtype=np.float32) * 0.01,
    }
    out = kernel(**ins)
    print("out", out.shape, out.dtype, np.abs(out).mean())
